# revision 1
# baseline (speedup 1.0000x reference)
"""DDSP core synthesizer kernel for Trainium2 (8 NeuronCores, data-parallel).

Reference computation (per row of B=32, T=64000):
    f0_hz = 20*exp(f0); phase = cumsum(2*pi*f0_hz/SR)
    hw    = sum_k sin(phase*k)/k   (k = 1..60)
    audio = mix*hw*loud + (1-mix)*noise*loud;  out = audio / (max|audio| + 1e-6)

Device algorithm (everything in "turns" = phase/2pi):
    inc  = exp(f0 + ln(20/SR))                       [ACT Exp]
    u    = blocked cumsum of inc                      [DVE scan + PE triangular mm]
    u1   = u - rint(u)                                [DVE magic-number rint]
    u_hi = f16(u1); u_lo = f16((u1-u_hi)*4096)        [exact 2-term split]
    per harmonic block (60 k's x 2 time-blocks on 120 partitions):
        x  = k*u_hi + (k/4096)*u_lo                   [PE f16 matmul, exact]
        r  = rint(x)                                  [DVE dual-op tensor_scalar]
        v  = x - r   in [-0.5, 0.5]                   [PE -identity @ r accumulate]
        s  = sin(2*pi*v)                              [ACT Sin, scale=2pi]
        hw += (1/k)^T @ s                             [PE fp32r matmul]
    epilogue: audio = loud*(noise + mix*(hw - noise)); peak-normalize per row
              (row max via free-reduce + DVE 32x32 transpose trick).

Sharding: pure data parallel, 4 rows per core, SPMD on cores 0-7.
"""

import sys

sys.path.insert(0, "/opt/trn_rl_repo")

import numpy as np
import ml_dtypes
from contextlib import ExitStack

import concourse.bass as bass
import concourse.tile as tile
from concourse import bacc, mybir
from concourse import bass_utils

f32 = np.float32
dt = mybir.dt

SR = 44100.0
H = 60                      # harmonics
B, T = 32, 64000
NCORES = 8
RPC = B // NCORES           # rows per core = 4
P = 128                     # SBUF partitions
FD = T * RPC // P           # free dim of master tiles = 2000
BPR = P // RPC              # blocks per row = 32
NPAIR = P // 2              # block pairs = 64
PI = float(np.pi)
MAGIC = float(1.5 * 2.0 ** 23)
LO_SCALE = 4096.0
Q_OFFS = [0, 512, 1024, 1536]
Q_LENS = [512, 512, 512, 464]
EXP_BIAS = float(np.log(20.0 / SR))

_cache = {}


def _consts():
    # lt: exclusive-prefix matmul weights. offs[m] = sum_k lt[k, m] * totals[k]
    kk, mm_ = np.meshgrid(np.arange(P), np.arange(P), indexing="ij")
    lt = ((kk // BPR == mm_ // BPR) & (kk % BPR < mm_ % BPR)).astype(f32)

    # Stage-2 partitioning: per pass, 64 local blocks x 2 harmonics fill
    # 128 partitions (p = 2*b_loc + kap, k = 2*kg + kap + 1; kg = 0..29).
    # Group tile uhalf[g] holds local block b at partitions 2b (hi), 2b+1 (lo).
    # xsel[kg]: lhsT [128, 128], x[2b+kap] = k*u_hi[b] + (k/4096)*u_lo[b].
    # wsel[kg]: lhsT [128, 64], hw[b] += sum_kap (1/k) * s[2b+kap].
    xsel = np.zeros((30, P, P), dtype=np.float64)
    wsel = np.zeros((30, P, 64), dtype=f32)  # cast at return
    negi = np.zeros((P, P), dtype=np.float64)
    for p in range(P):
        negi[p, p] = -1.0
    for kg in range(30):
        for b in range(64):
            for kap in range(2):
                k = 2 * kg + kap + 1.0
                xsel[kg, 2 * b + 0, 2 * b + kap] = k
                xsel[kg, 2 * b + 1, 2 * b + kap] = k / LO_SCALE
                wsel[kg, 2 * b + kap, b] = 1.0 / k
    xsel = xsel.astype(np.float16)
    negi = negi.astype(np.float16)
    wsel = wsel.astype(ml_dtypes.bfloat16)
    return {"lt": lt, "xsel": xsel, "wsel": wsel, "negi": negi}


def _build(xchunk=512, xbufs=6, sbufs=6, hsec=4, reps=1, act_pct=0, pool_epi=True, hbufs=2):
    nc = bacc.Bacc("TRN2", target_bir_lowering=False, debug=False,
                   enable_asserts=True, num_devices=NCORES)

    f0_d = nc.dram_tensor("f0", [P, FD], dt.float32, kind="ExternalInput")
    loud_d = nc.dram_tensor("loud", [P, FD], dt.float32, kind="ExternalInput")
    mix_d = nc.dram_tensor("mix", [P, FD], dt.float32, kind="ExternalInput")
    noise_d = nc.dram_tensor("noise", [P, FD], dt.float32, kind="ExternalInput")
    lt_d = nc.dram_tensor("lt", [P, P], dt.float32, kind="ExternalInput")
    xsel_d = nc.dram_tensor("xsel", [30, P, P], dt.float16, kind="ExternalInput")
    wsel_d = nc.dram_tensor("wsel", [30, P, 64], dt.bfloat16, kind="ExternalInput")
    negi_d = nc.dram_tensor("negi", [P, P], dt.float16, kind="ExternalInput")
    out_d = nc.dram_tensor("audio", [P, FD], dt.float32, kind="ExternalOutput")

    AF = mybir.ActivationFunctionType
    ALU = mybir.AluOpType

    with tile.TileContext(nc) as tc, ExitStack() as ctx:
        pool = ctx.enter_context(tc.tile_pool(name="sb", bufs=1))
        spool = ctx.enter_context(tc.tile_pool(name="sin", bufs=sbufs))
        rpool = ctx.enter_context(tc.tile_pool(name="rint", bufs=sbufs))
        xpool = ctx.enter_context(tc.tile_pool(name="xps", bufs=xbufs, space="PSUM"))
        hpool = ctx.enter_context(tc.tile_pool(name="hps", bufs=hbufs, space="PSUM"))

        def const_col(val, tag):
            t = pool.tile([P, 1], dt.float32, tag=tag)
            nc.vector.memset(t[:], val)
            return t

        exp_bias = const_col(EXP_BIAS, "cbias_exp")
        zero_bias = const_col(0.0, "cbias_zero")
        mag_bias = const_col(MAGIC, "cbias_mag")
        nmag_bias = const_col(-MAGIC, "cbias_nmag")

        # ---- input DMA ----
        f0 = pool.tile([P, FD], dt.float32, tag="scr", bufs=4, name="f0")
        nc.sync.dma_start(f0[:], f0_d.ap())
        lt = pool.tile([P, P], dt.float32)
        nc.gpsimd.dma_start(lt[:], lt_d.ap())
        xsel = pool.tile([P, 30, P], dt.float16)
        nc.gpsimd.dma_start(xsel[:], xsel_d.ap().rearrange("g p m -> p g m"))
        wsel = pool.tile([P, 30, 64], dt.bfloat16)
        nc.gpsimd.dma_start(wsel[:], wsel_d.ap().rearrange("g p m -> p g m"))
        negi = pool.tile([P, P], dt.float16)
        nc.gpsimd.dma_start(negi[:], negi_d.ap())
        # ---- stage 1: phase accumulation (turns) ----
        inc = pool.tile([P, FD], dt.float32, tag="scr", bufs=4, name="inc")
        nc.scalar.activation(inc[:], f0[:], AF.Exp, bias=exp_bias[:, 0:1], scale=1.0)

        local = pool.tile([P, FD], dt.float32, tag="scr", bufs=4, name="local")
        nc.vector.tensor_tensor_scan(local[:], inc[:], inc[:], 0.0,
                                     ALU.add, ALU.bypass)

        offs_ps = xpool.tile([P, 1], dt.float32, tag="x")
        nc.tensor.matmul(offs_ps[:], lt[:], local[:, FD - 1:FD],
                         start=True, stop=True)
        offs = pool.tile([P, 1], dt.float32)
        nc.vector.tensor_copy(offs[:], offs_ps[:])

        u = pool.tile([P, FD], dt.float32, tag="scr", bufs=4, name="u")
        nc.vector.tensor_scalar(u[:], local[:], offs[:, 0:1], None, ALU.add)
        # u1 = u - rint(u)  (safe: k*(u - n) == k*u mod 1)
        ur = pool.tile([P, FD], dt.float32, tag="scr", bufs=4, name="ur")
        nc.vector.tensor_scalar(ur[:], u[:], MAGIC, MAGIC, ALU.add, ALU.subtract)
        u1 = pool.tile([P, FD], dt.float32, tag="scr", bufs=4, name="u1")
        nc.gpsimd.tensor_tensor(u1[:], u[:], ur[:], ALU.subtract)

        # ---- f16 hi/lo split ----
        uhi = pool.tile([P, FD], dt.float16)
        nc.vector.tensor_copy(uhi[:], u1[:])
        ulo_f32 = pool.tile([P, FD], dt.float32, tag="scr", bufs=4, name="ulo_f32")
        nc.gpsimd.tensor_tensor(ulo_f32[:], u1[:], uhi[:], ALU.subtract)
        ulo = pool.tile([P, FD], dt.float16)
        nc.vector.tensor_scalar(ulo[:], ulo_f32[:], LO_SCALE, None, ALU.mult)

        # ---- repack: group tile g holds local block b=0..63 (global 64g+b)
        # at partitions 2b (hi) and 2b+1 (lo) ----
        uhi_v = uhi[:].rearrange("(g b) f -> g b f", g=2)
        ulo_v = ulo[:].rearrange("(g b) f -> g b f", g=2)
        uhalf = []
        for g in range(2):
            t = pool.tile([P, FD], dt.float16, tag=f"uhl{g}")
            tv = t[:].rearrange("(b s) f -> b s f", s=2)
            nc.sync.dma_start(tv[:, 0, :], uhi_v[g])
            nc.scalar.dma_start(tv[:, 1, :], ulo_v[g])
            uhalf.append(t)

        # epilogue inputs (issued after phase DMAs so they don't compete
        # for HBM bandwidth on the critical path)
        louds, mixs, noises = [], [], []
        for g in range(2):
            lg = pool.tile([64, FD], dt.float32, tag=f"loud{g}")
            nc.scalar.dma_start(lg[:], loud_d.ap()[64 * g:64 * g + 64, :])
            louds.append(lg)
            mg = pool.tile([64, FD], dt.float32, tag=f"mix{g}")
            nc.scalar.dma_start(mg[:], mix_d.ap()[64 * g:64 * g + 64, :])
            mixs.append(mg)
            ng = pool.tile([64, FD], dt.float32, tag=f"noise{g}")
            nc.scalar.dma_start(ng[:], noise_d.ap()[64 * g:64 * g + 64, :])
            noises.append(ng)

        # ---- stage 2 + 3: harmonic bank and epilogue, per block-group ----
        rep_ctx = tc.For_i(0, reps, 1) if reps > 1 else None
        if rep_ctx is not None:
            rep_ctx.__enter__()
        # hw_g[b_loc, t] (b_loc = 0..63, global block 64g + b_loc) accumulates
        # over 30 kg-passes at PSUM partitions 0..63 (col-group 3 of the PE
        # array cannot be a tiled matmul destination, so both groups write
        # at base 0 and the epilogue runs per group). The free dim is
        # processed in `hsec` sections to free PSUM banks for x buffers.
        pass_ctr = [0]
        sec_bounds = []
        spos = 0
        nbank = 4 // hsec
        for si in range(hsec):
            slen = min(nbank * 512, FD - spos)
            sec_bounds.append((spos, slen))
            spos += slen
        audio0 = pool.tile([64, FD], dt.float32, tag="audio0", name="audio0")
        audio1 = pool.tile([64, FD], dt.float32, tag="audio1", name="audio1")
        audios = [audio0, audio1]
        for (s0, sn_) in sec_bounds:
            for g in range(2):
                audio = audios[g]
                hw_g = hpool.tile([64, nbank, 512], dt.float32, tag="hw")
                chunks = []
                c0 = s0
                while c0 < s0 + sn_:
                    cn = min(xchunk, s0 + sn_ - c0)
                    chunks.append((c0, cn))
                    c0 += cn
                for (c0, cn) in chunks:
                    for kg in range(30):
                        x_ps = xpool.tile([P, xchunk], dt.float32, tag="x")
                        # x = k*u_hi + (k/4096)*u_lo  (exact f16 products)
                        for qo in range(0, cn, 512):
                            qn = min(512, cn - qo)
                            nc.tensor.matmul(
                                x_ps[:, qo:qo + qn],
                                xsel[:, kg, :],
                                uhalf[g][:, c0 + qo:c0 + qo + qn],
                                start=True, stop=False)
                        r = rpool.tile([P, xchunk], dt.float16, tag="r")
                        pass_ctr[0] += 1
                        if (pass_ctr[0] * act_pct) // 100 != ((pass_ctr[0] - 1) * act_pct) // 100:
                            # rint on ACT: t = Id(x + M); r = Id(t - M)
                            tti = spool.tile([P, xchunk], dt.float32, tag="ti")
                            nc.scalar.activation(tti[:, 0:cn], x_ps[:, 0:cn],
                                                 AF.Identity,
                                                 bias=mag_bias[:, 0:1], scale=1.0)
                            nc.scalar.activation(r[:, 0:cn], tti[:, 0:cn],
                                                 AF.Identity,
                                                 bias=nmag_bias[:, 0:1], scale=1.0)
                        else:
                            nc.vector.tensor_scalar(r[:, 0:cn], x_ps[:, 0:cn],
                                                    MAGIC, MAGIC,
                                                    ALU.add, ALU.subtract)
                        for qo in range(0, cn, 512):
                            qn = min(512, cn - qo)
                            nc.tensor.matmul(
                                x_ps[:, qo:qo + qn], negi[:], r[:, qo:qo + qn],
                                start=False, stop=True)
                        s = spool.tile([P, xchunk], dt.bfloat16, tag="s")
                        nc.scalar.activation(s[:, 0:cn], x_ps[:, 0:cn], AF.Sin,
                                             bias=zero_bias[:, 0:1],
                                             scale=2.0 * PI)
                        for qo in range(0, cn, 512):
                            qn = min(512, cn - qo)
                            q = (c0 + qo - s0) // 512
                            go = (c0 + qo - s0) % 512
                            nc.tensor.matmul(
                                hw_g[:, q, go:go + qn],
                                wsel[:, kg, :], s[:, qo:qo + qn],
                                start=(kg == 0), stop=(kg == 29))

                # audio = loud*(noise + mix*(hw - noise)) for this section
                hw_flat = hw_g[:].rearrange("p q f -> p (q f)")[:, 0:sn_]
                sl = slice(s0, s0 + sn_)
                e1 = pool.tile([64, FD], dt.float32, tag="escr", bufs=3, name=f"e1_{g}")
                nc.vector.tensor_tensor(e1[:, sl], hw_flat,
                                        noises[g][:, sl], ALU.subtract)
                epi_eng = nc.gpsimd if pool_epi else nc.vector
                e2 = pool.tile([64, FD], dt.float32, tag="escr", bufs=3, name=f"e2_{g}")
                epi_eng.tensor_tensor(e2[:, sl], e1[:, sl],
                                      mixs[g][:, sl], ALU.mult)
                e3 = pool.tile([64, FD], dt.float32, tag="escr", bufs=3, name=f"e3_{g}")
                epi_eng.tensor_tensor(e3[:, sl], e2[:, sl],
                                      noises[g][:, sl], ALU.add)
                epi_eng.tensor_tensor(audio[:, sl], e3[:, sl],
                                      louds[g][:, sl], ALU.mult)

        for g in range(2):
            audio = audios[g]
            # per-row peak: free-dim abs-max, then 32x32 transpose trick
            pk = pool.tile([64, 1], dt.float32, tag="pk")
            nc.vector.tensor_reduce(pk[:], audio[:], axis=mybir.AxisListType.X,
                                    op=ALU.max, apply_absolute_value=True)
            pkr = pool.tile([64, 32], dt.float32, tag="pkr")
            nc.vector.tensor_copy(pkr[:], pk[:, 0:1].to_broadcast((64, 32)))
            pkt = pool.tile([64, 32], dt.float32, tag="pkt")
            nc.vector.transpose(pkt[:], pkr[:])
            rowmax = pool.tile([64, 1], dt.float32, tag="rowmax")
            nc.vector.tensor_reduce(rowmax[:], pkt[:],
                                    axis=mybir.AxisListType.X, op=ALU.max)
            pke = pool.tile([64, 1], dt.float32, tag="pke")
            nc.vector.tensor_scalar(pke[:], rowmax[:], 1e-6, None, ALU.add)
            rcp = pool.tile([64, 1], dt.float32, tag="rcp")
            nc.vector.reciprocal(rcp[:], pke[:])
            outt = pool.tile([64, FD], dt.float32, tag="escr", bufs=3, name="outt")
            nc.vector.tensor_scalar(outt[:], audio[:], rcp[:, 0:1],
                                    None, ALU.mult)
            nc.sync.dma_start(out_d.ap()[64 * g:64 * g + 64, :], outt[:])

        if rep_ctx is not None:
            rep_ctx.__exit__(None, None, None)

    nc.compile()
    return nc


def kernel(f0, loudness, harmonic_mix, noise):
    if "nc" not in _cache:
        _cache["nc"] = _build()
        _cache["consts"] = _consts()
    nc = _cache["nc"]
    consts = _cache["consts"]

    def shard(a, c):
        return np.ascontiguousarray(
            a[c * RPC:(c + 1) * RPC].astype(f32, copy=False).reshape(P, FD))

    in_maps = []
    for c in range(NCORES):
        in_maps.append({
            "f0": shard(f0, c),
            "loud": shard(loudness, c),
            "mix": shard(harmonic_mix, c),
            "noise": shard(noise, c),
            **consts,
        })

    res = bass_utils.run_bass_kernel_spmd(nc, in_maps, core_ids=list(range(NCORES)))
    outs = [res.results[c]["audio"].reshape(RPC, T) for c in range(NCORES)]
    return np.concatenate(outs, axis=0)



# revision 2
# speedup vs baseline: 1.4257x; 1.4257x over previous
"""DDSP core synthesizer kernel for Trainium2 (8 NeuronCores, data-parallel).

Reference computation (per row of B=32, T=64000):
    f0_hz = 20*exp(f0); phase = cumsum(2*pi*f0_hz/SR)
    hw    = sum_k sin(phase*k)/k   (k = 1..60)
    audio = mix*hw*loud + (1-mix)*noise*loud;  out = audio / (max|audio| + 1e-6)

Device algorithm (phase in "turns"; harmonics in Q32 int fixed-point):
    inc  = exp(f0 + ln(20/SR))                       [ACT Exp]
    u    = blocked cumsum of inc                      [DVE scan + PE triangular mm]
    u1   = u - rint(u)  in [-0.5, 0.5]                [DVE magic rint + tt subtract]
    per harmonic k (flat layout: 128 blocks x 2000):
        v_k = k*u1*2^32 mod 2^32  (int32, exact wrap-around phase)
          k=1:    v_1 = int32(u1 * 2^32)              [DVE mult]
          even:   v_2m = v_m << 1                     [DVE arith_shift_left, wraps]
          odd:    w = int32(u1 * k*2^26); v = w << 6  [DVE mult + shift]
            or    v_k = v_{k-2} + v_2 (mod 2^32)      [GpSimd tt add, wraps]
        s_k = sin(2*pi*2^-32 * v_k)  -> f16           [ACT Sin, int32 input]
        hw += diag(1/k) @ s_k                         [PE f16 matmul, PSUM accum]
    epilogue: audio = B*hw + A with B = loud*mix, A = noise*(loud-B)
              peak-normalize per row (free abs-max + 32x32 transpose trick).

Sharding: pure data parallel, 4 rows per core, SPMD on cores 0-7.
"""

import sys

sys.path.insert(0, "/opt/trn_rl_repo")

import numpy as np
import ml_dtypes
from contextlib import ExitStack

import concourse.bass as bass
import concourse.tile as tile
from concourse import bacc, mybir
from concourse import bass_utils

f32 = np.float32
dt = mybir.dt

SR = 44100.0
H = 60                      # harmonics
B, T = 32, 64000
NCORES = 8
RPC = B // NCORES           # rows per core = 4
P = 128                     # SBUF partitions
FD = T * RPC // P           # free dim of master tiles = 2000
BPR = P // RPC              # blocks per row = 32
PI = float(np.pi)
MAGIC = float(1.5 * 2.0 ** 23)
EXP_BIAS = float(np.log(20.0 / SR))
Q26 = float(2.0 ** 26)
Q32 = float(2.0 ** 32)

# harmonic processing order: odd heads ascending, each head's doubling chain
def _chains():
    order = []
    for h in range(1, H + 1, 2):
        k = h
        while k <= H:
            order.append(k)
            k *= 2
    return order

_cache = {}


def _consts():
    # lt: exclusive-prefix matmul weights. offs[m] = sum_k lt[k, m] * totals[k]
    kk, mm_ = np.meshgrid(np.arange(P), np.arange(P), indexing="ij")
    lt = ((kk // BPR == mm_ // BPR) & (kk % BPR < mm_ % BPR)).astype(f32)

    # diag[k-1] = (1/k) * I, f16, used as lhsT of the hw accumulation
    diag = np.zeros((H, P, P), dtype=np.float16)
    for k in range(1, H + 1):
        diag[k - 1, np.arange(P), np.arange(P)] = np.float16(1.0 / k)
    return {"lt": lt, "diag": diag}


def _build(gp_heads=15, vbufs=6, sbufs=5):
    nc = bacc.Bacc("TRN2", target_bir_lowering=False, debug=False,
                   enable_asserts=True, num_devices=NCORES)

    f0_d = nc.dram_tensor("f0", [P, FD], dt.float32, kind="ExternalInput")
    loud_d = nc.dram_tensor("loud", [P, FD], dt.float32, kind="ExternalInput")
    mix_d = nc.dram_tensor("mix", [P, FD], dt.float32, kind="ExternalInput")
    noise_d = nc.dram_tensor("noise", [P, FD], dt.float32, kind="ExternalInput")
    lt_d = nc.dram_tensor("lt", [P, P], dt.float32, kind="ExternalInput")
    diag_d = nc.dram_tensor("diag", [H, P, P], dt.float16, kind="ExternalInput")
    out_d = nc.dram_tensor("audio", [P, FD], dt.float32, kind="ExternalOutput")

    AF = mybir.ActivationFunctionType
    ALU = mybir.AluOpType

    # odd heads >= 3 computed on GpSimd via wrapping add (v_k = v_{k-2} + v_2)
    gp_odd = set(range(3, 3 + 2 * gp_heads, 2)) & set(range(3, H, 2))

    with tile.TileContext(nc) as tc, ExitStack() as ctx:
        pool = ctx.enter_context(tc.tile_pool(name="sb", bufs=1))
        vpool = ctx.enter_context(tc.tile_pool(name="vp", bufs=vbufs))
        spool = ctx.enter_context(tc.tile_pool(name="sp", bufs=sbufs))
        xpool = ctx.enter_context(tc.tile_pool(name="xps", bufs=1, space="PSUM"))
        hpool = ctx.enter_context(tc.tile_pool(name="hps", bufs=1, space="PSUM"))

        def const_col(val, tag):
            t = pool.tile([P, 1], dt.float32, tag=tag)
            nc.vector.memset(t[:], val)
            return t

        exp_bias = const_col(EXP_BIAS, "cbias_exp")
        zero_bias = const_col(0.0, "cbias_zero")

        # ---- input DMA ----
        f0 = pool.tile([P, FD], dt.float32, tag="scr", bufs=4, name="f0")
        nc.sync.dma_start(f0[:], f0_d.ap())
        lt = pool.tile([P, P], dt.float32)
        nc.gpsimd.dma_start(lt[:], lt_d.ap())
        diag = pool.tile([P, H, P], dt.float16)
        nc.gpsimd.dma_start(diag[:], diag_d.ap().rearrange("g p m -> p g m"))
        loud = pool.tile([P, FD], dt.float32, tag="loud")
        nc.scalar.dma_start(loud[:], loud_d.ap())
        mix = pool.tile([P, FD], dt.float32, tag="mix")
        nc.scalar.dma_start(mix[:], mix_d.ap())
        noise = pool.tile([P, FD], dt.float32, tag="noise")
        nc.scalar.dma_start(noise[:], noise_d.ap())

        # ---- stage 1: phase accumulation (turns) ----
        inc = pool.tile([P, FD], dt.float32, tag="scr", bufs=4, name="inc")
        nc.scalar.activation(inc[:], f0[:], AF.Exp, bias=exp_bias[:, 0:1], scale=1.0)

        local = pool.tile([P, FD], dt.float32, tag="scr", bufs=4, name="local")
        nc.vector.tensor_tensor_scan(local[:], inc[:], inc[:], 0.0,
                                     ALU.add, ALU.bypass)

        offs_ps = xpool.tile([P, 1], dt.float32, tag="x")
        nc.tensor.matmul(offs_ps[:], lt[:], local[:, FD - 1:FD],
                         start=True, stop=True)
        offs = pool.tile([P, 1], dt.float32)
        nc.vector.tensor_copy(offs[:], offs_ps[:])

        u = pool.tile([P, FD], dt.float32, tag="scr", bufs=4, name="u")
        nc.vector.tensor_scalar(u[:], local[:], offs[:, 0:1], None, ALU.add)
        # u1 = u - rint(u) in [-0.5, 0.5]
        ur = pool.tile([P, FD], dt.float32, tag="scr", bufs=4, name="ur")
        nc.vector.tensor_scalar(ur[:], u[:], MAGIC, MAGIC, ALU.add, ALU.subtract)
        u1 = pool.tile([P, FD], dt.float32, tag="u1")
        nc.vector.tensor_tensor(u1[:], u[:], ur[:], ALU.subtract)

        # ---- stage 2: harmonic bank, flat layout ----
        hw_ps = hpool.tile([P, 4, 512], dt.float32, tag="hw")
        chunks = []
        c0 = 0
        while c0 < FD:
            cn = min(512, FD - c0)
            chunks.append((c0, cn))
            c0 += cn

        order = _chains()
        vmap = {}          # k -> live v tile
        v2_res = pool.tile([P, FD], dt.int32, tag="v2res")   # v_2 resident
        prev_odd = [None]  # previous odd-head tile (for gp chain)

        # epilogue precompute placed mid-stage on gpsimd:
        #   Bm = loud*mix ; Am = loud - Bm ; A = noise*Am
        Bm = pool.tile([P, FD], dt.float32, tag="Bm")
        Am = pool.tile([P, FD], dt.float32, tag="Am")
        A = pool.tile([P, FD], dt.float32, tag="A")
        epi_at = {order[min(len(order) - 1, 8)]: 0}

        emitted_epi = [False]

        def emit_epi():
            nc.gpsimd.tensor_tensor(Bm[:], loud[:], mix[:], ALU.mult)
            nc.gpsimd.tensor_tensor(Am[:], loud[:], Bm[:], ALU.subtract)
            nc.gpsimd.tensor_tensor(A[:], noise[:], Am[:], ALU.mult)
            emitted_epi[0] = True

        first_k = order[0]
        last_k = order[-1]
        for ki, k in enumerate(order):
            if k in epi_at and not emitted_epi[0]:
                emit_epi()
            # ---- produce v_k (int32 Q32 phase) ----
            if k == 1:
                v = vpool.tile([P, FD], dt.int32, tag="v")
                nc.vector.tensor_scalar(v[:], u1[:], Q32, None, ALU.mult)
            elif k % 2 == 0:
                src = vmap[k // 2]
                if k == 2:
                    v = v2_res
                else:
                    v = vpool.tile([P, FD], dt.int32, tag="v")
                nc.vector.tensor_scalar(v[:], src[:], 1, None,
                                        ALU.arith_shift_left)
            elif k in gp_odd and prev_odd[0] is not None:
                v = vpool.tile([P, FD], dt.int32, tag="v")
                nc.gpsimd.tensor_tensor(v[:], prev_odd[0][:], v2_res[:], ALU.add)
            else:
                w = vpool.tile([P, FD], dt.int32, tag="v")
                nc.vector.tensor_scalar(w[:], u1[:], float(k) * Q26, None,
                                        ALU.mult)
                v = vpool.tile([P, FD], dt.int32, tag="v")
                nc.vector.tensor_scalar(v[:], w[:], 6, None,
                                        ALU.arith_shift_left)
            vmap[k] = v
            if k % 2 == 1:
                prev_odd[0] = v

            # ---- s_k = sin(2*pi*2^-32 * v_k), f16 ----
            s = spool.tile([P, FD], dt.float16, tag="s")
            nc.scalar.activation(s[:], v[:], AF.Sin, bias=zero_bias[:, 0:1],
                                 scale=float(2.0 * PI / Q32))

            # ---- hw += (1/k) * s ----
            for q, (c0, cn) in enumerate(chunks):
                nc.tensor.matmul(hw_ps[:, q, 0:cn], diag[:, k - 1, :],
                                 s[:, c0:c0 + cn],
                                 start=(k == first_k), stop=(k == last_k))

        if not emitted_epi[0]:
            emit_epi()

        # ---- epilogue: audio = Bm*hw + A, then peak-normalize per row ----
        hw_flat = hw_ps[:].rearrange("p q f -> p (q f)")[:, 0:FD]
        t1 = pool.tile([P, FD], dt.float32, tag="t1")
        nc.vector.tensor_tensor(t1[:], hw_flat, Bm[:], ALU.mult)
        audio = pool.tile([P, FD], dt.float32, tag="audio")
        nc.vector.tensor_tensor(audio[:], t1[:], A[:], ALU.add)

        # per-row peak: free-dim abs-max then 32x32 transpose trick
        pk = pool.tile([P, 1], dt.float32, tag="pk")
        nc.vector.tensor_reduce(pk[:], audio[:], axis=mybir.AxisListType.X,
                                op=ALU.max, apply_absolute_value=True)
        pkr = pool.tile([P, 32], dt.float32, tag="pkr")
        nc.vector.tensor_copy(pkr[:], pk[:, 0:1].to_broadcast((P, 32)))
        pkt = pool.tile([P, 32], dt.float32, tag="pkt")
        nc.vector.transpose(pkt[:], pkr[:])
        rowmax = pool.tile([P, 1], dt.float32, tag="rowmax")
        nc.vector.tensor_reduce(rowmax[:], pkt[:],
                                axis=mybir.AxisListType.X, op=ALU.max)
        pke = pool.tile([P, 1], dt.float32, tag="pke")
        nc.vector.tensor_scalar(pke[:], rowmax[:], 1e-6, None, ALU.add)
        rcp = pool.tile([P, 1], dt.float32, tag="rcp")
        nc.vector.reciprocal(rcp[:], pke[:])
        outt = pool.tile([P, FD], dt.float32, tag="outt")
        nc.vector.tensor_scalar(outt[:], audio[:], rcp[:, 0:1], None, ALU.mult)
        nc.sync.dma_start(out_d.ap(), outt[:])

    nc.compile()
    return nc


def kernel(f0, loudness, harmonic_mix, noise):
    if "nc" not in _cache:
        _cache["nc"] = _build()
        _cache["consts"] = _consts()
    nc = _cache["nc"]
    consts = _cache["consts"]

    def shard(a, c):
        return np.ascontiguousarray(
            a[c * RPC:(c + 1) * RPC].astype(f32, copy=False).reshape(P, FD))

    in_maps = []
    for c in range(NCORES):
        in_maps.append({
            "f0": shard(f0, c),
            "loud": shard(loudness, c),
            "mix": shard(harmonic_mix, c),
            "noise": shard(noise, c),
            **consts,
        })

    res = bass_utils.run_bass_kernel_spmd(nc, in_maps, core_ids=list(range(NCORES)))
    outs = [res.results[c]["audio"].reshape(RPC, T) for c in range(NCORES)]
    return np.concatenate(outs, axis=0)
